# revision 44
# baseline (speedup 1.0000x reference)
"""Multi-head attention (B=2, S=4096, D=512, H=8) on 8 Trainium2 NeuronCores.

Sharding: batch x head-pair parallelism. Core c handles batch b = c // 4 and
heads {2*(c%4), 2*(c%4)+1} (128 contiguous rows of the QKV projection
weights, Megatron column-parallel; Wo row-parallel with the partial-sum
reduction done on the host at gather time).

Per-core device program (matmul operands bf16, accumulation fp32 PSUM).
Structured so the Scalar engine's exp stream (the throughput floor) is fed
continuously:
  - Fine-grained input DMA; K-block0/Q-block0/V-group0 projected up front;
    remaining K blocks / V groups / per-qblock Q projections are injected
    just-in-time into qb0's loop, so the first exp fires early and the
    Scalar engine never waits behind a serial projection phase.
  - Scores per (kt-pair, head): scoresT [128(k), 1024(q-pair)] f32 PSUM,
    two N=512 matmuls (PSUM bank limit); the K=64 contraction auto
    row-tiles (h0 on partitions 0-63, h1 on 64-127) so the two heads'
    score matmuls co-execute in the PE array.
  - exp: most tiles on ACT (PSUM->SBUF bf16, FD=1024); ktg%4==1 tiles are
    computed on DVE via a Schraudolph bit-hack (int16(x*128/ln2 + 16251)
    bitcast bf16, ~3% max rel err) to share the exp load across engines.
    DVE slots sit away from qblock boundaries so the normalization chain
    never queues behind them.
  - AV: vh tiles [128, 65] (65th col = ones -> softmax denominator) as
    stationary; accumulated over all 32 key tiles directly in PSUM
    ([65, 512] per head, held across the qblock), one ktg behind scores.
  - Normalize (deferred into the next qblock): denominator row moved to a
    q-on-partitions layout with tiny K=1 matmuls, ONE [128, 8] exact
    reciprocal (128-lane parallel), per-head output projection, 1/den
    applied as a per-partition tensor_scalar at evacuation, heads summed
    on DVE, bf16 partials DMA'd out. norm_a runs right after the qblock's
    last AV matmul (releasing the av PSUM banks); out-projection subtiles
    spread over even ktg slots of the next qblock.

Host gathers: out[b] = sum of the 4 per-core partials + bv @ Wo.T + bo.
"""

from collections import defaultdict

import ml_dtypes
import numpy as np

import concourse.mybir as mybir
import concourse.tile as tile
from concourse import bacc
from concourse.bass_utils import run_bass_kernel_spmd

F32 = mybir.dt.float32
BF16 = mybir.dt.bfloat16
I16 = mybir.dt.int16
EXP = mybir.ActivationFunctionType.Exp
ADD = mybir.AluOpType.add
MULT = mybir.AluOpType.mult
NPBF16 = ml_dtypes.bfloat16

B, S, D, H = 2, 4096, 512, 8
DK = D // H          # 64
HPC = 2              # heads per core
HD = HPC * DK        # 128 head-dims per core
N_CORES = 8
QB = 512             # query block (matmul free dim)
KT = 128             # key tile (partition dim)
NCH = D // 128       # 4 contraction chunks for the projections
KPG = 2              # key tiles per score/exp group
SCW = KPG * QB       # score tile width (1024)
NSUB = QB // 128     # out-projection subtiles per qblock (4)

# Schraudolph bf16 exp: exp(s) ~= bitcast_bf16(int16(s*C1 + C2)); C1 = 2^7/ln2,
# C2 = 127*128 - sigma with sigma tuned for truncating f32->int16 conversion.
EXP_C1 = 184.6649652337873
EXP_C2 = 16251.0
# ktg slots whose exp runs on DVE instead of ACT: ktg % DVE_MOD == 1
DVE_MOD = 4


def mha_tile_kernel(tc, out_ap, ins, seq=S, dve_mod=DVE_MOD):
    nc = tc.nc
    nqb, nkt = seq // QB, seq // KT
    nktg = nkt // KPG
    nst = seq // 128                      # 128-wide s-subtiles for V
    vgroups = [range(g, min(g + 8, nst)) for g in range(0, nst, 8)]

    xq, xk, xv = ins["qt"], ins["kt"], ins["vt"]
    const = tc.alloc_tile_pool(name="const", bufs=1)
    sb = tc.alloc_tile_pool(name="sb", bufs=2)
    scp = tc.alloc_tile_pool(name="scp", bufs=3, space="PSUM")
    avp = tc.alloc_tile_pool(name="avp", bufs=2, space="PSUM")

    # --- weights / constants ---
    wq_sb = const.tile([128, NCH, 128], BF16, tag="wq", name="wq_sb")
    wk_sb = const.tile([128, NCH, 128], BF16, tag="wk", name="wk_sb")
    wv_sb = const.tile([128, NCH, 128], BF16, tag="wv", name="wv_sb")
    # weights are host-permuted to [p, c, m] so this DMA is contiguous
    # (1KB/partition runs) instead of a slow 256B-strided gather
    for w_sb, name in ((wk_sb, "wk"), (wq_sb, "wq"), (wv_sb, "wv")):
        nc.sync.dma_start(w_sb, ins[name].rearrange("p (c m) -> p c m", m=128))
    wo0_sb = const.tile([64, QB], BF16, tag="wo0", name="wo0_sb")
    wo1_sb = const.tile([64, QB], BF16, tag="wo1", name="wo1_sb")
    nc.sync.dma_start(wo0_sb, ins["wo0"])
    nc.sync.dma_start(wo1_sb, ins["wo1"])
    bq_sb = const.tile([128, 1], F32, tag="bq", name="bq_sb")
    bk_sb = const.tile([128, 1], F32, tag="bk", name="bk_sb")
    nc.sync.dma_start(bq_sb, ins["bq"])
    nc.sync.dma_start(bk_sb, ins["bk"])
    ones_sb = const.tile([128, 64], F32, tag="ones", name="ones_sb")
    nc.vector.memset(ones_sb, 1.0)

    # --- persistent activations ---
    qhT = const.tile([128, seq], BF16, tag="qhT", name="qhT")
    khT = const.tile([128, seq], BF16, tag="khT", name="khT")
    vh = [
        const.tile([128, nkt * 65], BF16, tag=f"vh{h}", name=f"vh{h}")
        for h in range(HPC)
    ]
    for h in range(HPC):
        ones_col = vh[h].rearrange("p (j c) -> p j c", c=65)[:, :, 64]
        nc.vector.tensor_copy(out=ones_col, in_=ones_sb[:, 0:nkt])

    # --- raw inputs in SBUF; DMA'd in [128, 512] slices in consumption order
    xk_sb = [const.tile([128, seq], BF16, tag=f"xk{c}", name=f"xk{c}") for c in range(NCH)]
    xq_sb = [const.tile([128, seq], BF16, tag=f"xq{c}", name=f"xq{c}") for c in range(NCH)]
    xv_sb = [const.tile([128, seq], BF16, tag=f"xv{c}", name=f"xv{c}") for c in range(NCH)]

    def dma_x(dst_tiles, src, j):
        sl = slice(j * QB, (j + 1) * QB)
        for c in range(NCH):
            nc.sync.dma_start(dst_tiles[c][:, sl], src[c * 128 : (c + 1) * 128, sl])

    nvg0 = (len(vgroups[0]) + 3) // 4   # 512-slices covering v-group 0
    dma_x(xk_sb, xk, 0)
    dma_x(xq_sb, xq, 0)
    for j in range(nvg0):
        dma_x(xv_sb, xv, j)
    # remaining K blocks and V groups ordered by their first-use ktg slot in
    # qb0 (K block j injected at ktg 2j-2, V group g at ktg 4g-1), so the PE
    # never waits on an input slice that was queued behind later-needed data
    rest = []
    for j in range(1, nqb):
        rest.append((2 * j - 2, "k", [j]))
    for g in range(1, len(vgroups)):
        rest.append((4 * g - 1, "v", [s for s in (2 * g, 2 * g + 1) if s < nqb]))
    for _, kind, idxs in sorted(rest, key=lambda r: r[0]):
        for j in idxs:
            dma_x(xk_sb if kind == "k" else xv_sb, xk if kind == "k" else xv, j)
    for j in range(1, nqb):
        dma_x(xq_sb, xq, j)

    # --- projection emitters ---
    def proj_block(x_sb, w_sb, bias, dstT, j):
        qsl = slice(j * QB, (j + 1) * QB)
        acc = scp.tile([128, SCW], F32, tag="sc", name=f"prj{j}")[:, 0:QB]
        for c in range(NCH):
            nc.tensor.matmul(
                acc, lhsT=w_sb[:, c, :], rhs=x_sb[c][:, qsl],
                start=(c == 0), stop=(c == NCH - 1),
            )
        if bias is None:
            nc.vector.tensor_copy(out=dstT[:, qsl], in_=acc)
        else:
            nc.vector.tensor_scalar(dstT[:, qsl], acc, bias[:, 0:1], None, ADD)

    def vproj_group(g):
        sts = vgroups[g]
        acc = scp.tile([128, SCW], F32, tag="sc", name=f"vprj{g}")
        for i, st in enumerate(sts):
            a = acc[:, i * 128 : (i + 1) * 128]
            for c in range(NCH):
                nc.tensor.matmul(
                    a, lhsT=xv_sb[c][:, st * 128 : (st + 1) * 128],
                    rhs=wv_sb[:, c, :],
                    start=(c == 0), stop=(c == NCH - 1),
                )
        av_view = acc.rearrange("p (i m) -> p i m", m=128)[:, 0 : len(sts), :]
        for h in range(HPC):
            dst = vh[h].rearrange("p (j c) -> p j c", c=65)[
                :, sts[0] : sts[0] + len(sts), 0:64
            ]
            nc.vector.tensor_copy(out=dst, in_=av_view[:, :, h * 64 : (h + 1) * 64])

    # --- injection schedule: projections emitted inside qb0's ktg loop ---
    inj = defaultdict(list)
    for j in range(1, nqb):                       # K block j needed at ktg 2j
        inj[(0, max(0, min(2 * j - 2, nktg - 1)))].append(("K", j))
    for g in range(1, len(vgroups)):              # V group g needed at ktg 4g+1
        inj[(0, max(0, min(4 * g - 1, nktg - 1)))].append(("V", g))
    for qb in range(nqb - 1):                     # Q block qb+1 inside qb
        inj[(qb, max(0, nktg - 4))].append(("Q", qb + 1))

    # upfront: K block0, Q block0, V group0
    proj_block(xk_sb, wk_sb, bk_sb, khT, 0)
    proj_block(xq_sb, wq_sb, bq_sb, qhT, 0)
    vproj_group(0)

    def emit_av(av_t, ktg, ex):
        for part in range(KPG):
            kt_i = KPG * ktg + part
            for h in range(HPC):
                nc.tensor.matmul(
                    av_t[h][0:65, :],
                    lhsT=vh[h][:, kt_i * 65 : kt_i * 65 + 65],
                    rhs=ex[h][:, part * QB : (part + 1) * QB],
                    start=(ktg == 0 and part == 0),
                    stop=(ktg == nktg - 1 and part == KPG - 1),
                )

    # --- normalize + output projection (deferred into the next qblock) ---
    def norm_a(nstate):
        """Right after the qblock's last AV matmul: move denominators to a
        q-on-partitions layout (tiny K=1 matmuls), one wide reciprocal, and
        release the av PSUM banks via bf16 copies of the AV values."""
        av_t, st8 = nstate["av"], nstate["st8"]
        den_sb = sb.tile([128, SCW], F32, tag="den", bufs=2, name="den_sb")
        av_sb = []
        for h in range(HPC):
            nc.vector.tensor_copy(
                out=den_sb[64:65, h * QB : (h + 1) * QB], in_=av_t[h][64:65, :]
            )
            a = sb.tile([64, QB], BF16, tag=f"avs{h}", bufs=2, name=f"avs{h}")
            nc.vector.tensor_copy(out=a, in_=av_t[h][0:64, :])
            av_sb.append(a)
        denT = scp.tile([128, SCW], F32, tag="sc", name="denT")
        for h in range(HPC):
            for st in range(NSUB):
                j = h * NSUB + st
                o = h * QB + st * 128
                nc.tensor.matmul(
                    denT[:, j : j + 1],
                    lhsT=den_sb[64:65, o : o + 128],
                    rhs=ones_sb[64:65, 0:1],
                    start=True, stop=True,
                )
        rt_sb = sb.tile([128, 2 * NSUB], F32, tag="rt", bufs=2, name="rt_sb")
        nc.vector.reciprocal(
            out=rt_sb, in_=denT[:, 0 : 2 * NSUB]
        )
        st8["rt"] = rt_sb
        st8["av_sb"] = av_sb

    def norm_b(nstate):
        """One out-projection subtile: per-head K=64 matmuls into op halves,
        per-partition 1/den scale at evacuation, DVE head-sum, DMA out."""
        st = nstate["sub"]
        nstate["sub"] += 1
        qb, st8 = nstate["qb"], nstate["st8"]
        rt_sb, av_sb = st8["rt"], st8["av_sb"]
        ssl = slice(st * 128, (st + 1) * 128)
        op = scp.tile([128, SCW], F32, tag="sc", name="op")
        t_h = []
        for h in range(HPC):
            nc.tensor.matmul(
                op[:, h * QB : (h + 1) * QB],
                lhsT=av_sb[h][:, ssl],
                rhs=(wo0_sb if h == 0 else wo1_sb),
                start=True, stop=True,
            )
            t = sb.tile([128, QB], BF16, tag="ot", bufs=4, name=f"ot{h}")
            nc.vector.tensor_scalar(
                t, op[:, h * QB : (h + 1) * QB],
                rt_sb[:, h * NSUB + st : h * NSUB + st + 1], None, MULT,
            )
            t_h.append(t)
        ost = sb.tile([128, QB], BF16, tag="ost", bufs=3, name="ost")
        nc.vector.tensor_tensor(ost, t_h[0], t_h[1], ADD)
        base = qb * QB + st * 128
        nc.sync.dma_start(out_ap[base : base + 128, :], ost)

    # --- attention ---
    nstate = None
    for qb in range(nqb):
        qsl = slice(qb * QB, (qb + 1) * QB)
        av_t = [
            avp.tile([128, QB], F32, tag="av", name=f"av{h}") for h in range(HPC)
        ]
        prev = None
        for ktg in range(nktg):
            for kind, arg in inj.get((qb, ktg), ()):
                if kind == "K":
                    proj_block(xk_sb, wk_sb, bk_sb, khT, arg)
                elif kind == "Q":
                    proj_block(xq_sb, wq_sb, bq_sb, qhT, arg)
                else:
                    vproj_group(arg)
            # scores for both heads (row-tiled K=64 pairs co-execute)
            sc_t = [
                scp.tile([128, SCW], F32, tag="sc", name=f"sc{h}")
                for h in range(HPC)
            ]
            for part in range(KPG):
                kt_i = KPG * ktg + part
                ksl = slice(kt_i * KT, (kt_i + 1) * KT)
                for h in range(HPC):
                    hp = slice(h * 64, (h + 1) * 64)
                    nc.tensor.matmul(
                        sc_t[h][:, part * QB : (part + 1) * QB],
                        lhsT=khT[hp, ksl], rhs=qhT[hp, qsl],
                        start=True, stop=True,
                    )
            # exp: ACT or DVE (Schraudolph) per schedule
            ex = []
            for h in range(HPC):
                if dve_mod and ktg % dve_mod == 1:
                    e = sb.tile([128, SCW], I16, tag="exi", bufs=4, name="exi")
                    nc.vector.tensor_scalar(
                        e[:, :], sc_t[h], EXP_C1, EXP_C2, MULT, ADD
                    )
                    ex.append(e[:, :].bitcast(BF16))
                else:
                    e = sb.tile([128, SCW], BF16, tag="exb", bufs=5, name="exb")
                    nc.scalar.activation(e, sc_t[h], EXP)
                    ex.append(e[:, :])
            # deferred out-projection of the previous qblock: one subtile per
            # even ktg slot (DVE-exp slots are odd, so the evacuation work
            # never queues behind a DVE exp tile)
            if (
                nstate is not None
                and ktg >= 2
                and (ktg - 2) % 4 == 0
                and nstate["sub"] < NSUB
            ):
                norm_b(nstate)
            if prev is not None:
                emit_av(av_t, *prev)
            prev = (ktg, ex)
        emit_av(av_t, *prev)
        if nstate is not None:  # flush any remaining subtiles (small-seq case)
            while nstate["sub"] < NSUB:
                norm_b(nstate)
        # normalization prep right after this qblock's last AV matmul
        nstate = {"av": av_t, "qb": qb, "st8": {}, "sub": 0}
        norm_a(nstate)
    while nstate["sub"] < NSUB:
        norm_b(nstate)

    avp.release()
    scp.release()
    sb.release()
    const.release()


def build_bass(seq=S, dve_mod=DVE_MOD):
    nc = bacc.Bacc(
        "TRN2",
        debug=False,
        enable_asserts=False,
        target_bir_lowering=False,
    )
    ins = {}
    shapes = {
        "qt": (D, seq), "kt": (D, seq), "vt": (D, seq),
        "wq": (128, NCH * HD), "wk": (128, NCH * HD), "wv": (128, NCH * HD),
        "wo0": (64, D), "wo1": (64, D),
        "bq": (HD, 1), "bk": (HD, 1),
    }
    bf16_names = {"qt", "kt", "vt", "wq", "wk", "wv", "wo0", "wo1"}
    for name, shape in shapes.items():
        dt = BF16 if name in bf16_names else F32
        ins[name] = nc.dram_tensor(name, list(shape), dt, kind="ExternalInput").ap()
    out = nc.dram_tensor("out", [seq, D], BF16, kind="ExternalOutput").ap()
    with tile.TileContext(nc) as tc:
        mha_tile_kernel(tc, out, ins, seq=seq, dve_mod=dve_mod)
    nc.compile()
    return nc


def shard_inputs(q, k, v, Wq, bq, Wk, bk, Wv, bv, Wo, bo, seq=S):
    """Host-side shard prep. Returns (in_maps, const_vec)."""
    scale = 1.0 / np.sqrt(np.float32(DK))
    q, k, v = (np.asarray(x, np.float32) for x in (q, k, v))
    Wq, bq, Wk, bk, Wv, bv, Wo, bo = (
        np.asarray(x, np.float32) for x in (Wq, bq, Wk, bk, Wv, bv, Wo, bo)
    )
    bf = lambda x: np.ascontiguousarray(x).astype(NPBF16)
    # device weight layout [p, c*m]: w_dev[p, c, m] = w.T[c*128 + p, m]
    wperm = lambda w: (
        w.T.reshape(NCH, 128, HD).transpose(1, 0, 2).reshape(128, NCH * HD)
    )
    in_maps = []
    for c in range(N_CORES):
        b = c // 4
        rows = slice(128 * (c % 4), 128 * (c % 4) + 128)
        in_maps.append({
            "qt": bf(q[b].T),
            "kt": bf(k[b].T),
            "vt": bf(v[b].T),
            "wq": bf(wperm(Wq[rows, :] * scale)),
            "wk": bf(wperm(Wk[rows, :])),
            "wv": bf(wperm(Wv[rows, :])),
            "wo0": bf(Wo[:, rows][:, 0:64].T),
            "wo1": bf(Wo[:, rows][:, 64:128].T),
            "bq": np.ascontiguousarray((bq[rows] * scale).reshape(HD, 1)),
            "bk": np.ascontiguousarray(bk[rows].reshape(HD, 1)),
        })
    const_vec = (bv @ Wo.T + bo).astype(np.float32)
    return in_maps, const_vec


_NC_CACHE = {}


def run(inputs, seq=S, trace=False, trace_kwargs=None):
    if seq not in _NC_CACHE:
        _NC_CACHE[seq] = build_bass(seq=seq)
    nc = _NC_CACHE[seq]
    in_maps, const_vec = shard_inputs(**inputs, seq=seq)
    res = run_bass_kernel_spmd(
        nc,
        in_maps,
        core_ids=list(range(N_CORES)),
        trace=trace,
        **(trace_kwargs or {}),
    )
    out = np.zeros((B, seq, D), dtype=np.float32)
    for c in range(N_CORES):
        out[c // 4] += np.asarray(res.results[c]["out"], dtype=np.float32)
    out += const_vec[None, None, :]
    return out, res


def kernel(**inputs):
    out, _ = run(inputs)
    return out


# revision 46
# speedup vs baseline: 1.0074x; 1.0074x over previous
"""Multi-head attention (B=2, S=4096, D=512, H=8) on 8 Trainium2 NeuronCores.

Sharding: batch x head-pair parallelism. Core c handles batch b = c // 4 and
heads {2*(c%4), 2*(c%4)+1} (128 contiguous rows of the QKV projection
weights, Megatron column-parallel; Wo row-parallel with the partial-sum
reduction done on the host at gather time).

Per-core device program (matmul operands bf16, accumulation fp32 PSUM).
Structured so the Scalar engine's exp stream (the throughput floor) is fed
continuously:
  - Fine-grained input DMA; K-block0/Q-block0/V-group0 projected up front;
    remaining K blocks / V groups / per-qblock Q projections are injected
    just-in-time into qb0's loop, so the first exp fires early and the
    Scalar engine never waits behind a serial projection phase.
  - Scores per (kt-pair, head): scoresT [128(k), 1024(q-pair)] f32 PSUM,
    two N=512 matmuls (PSUM bank limit); the K=64 contraction auto
    row-tiles (h0 on partitions 0-63, h1 on 64-127) so the two heads'
    score matmuls co-execute in the PE array.
  - exp: most tiles on ACT (PSUM->SBUF bf16, FD=1024); ktg%4==1 tiles are
    computed on DVE via a Schraudolph bit-hack (int16(x*128/ln2 + 16251)
    bitcast bf16, ~3% max rel err) to share the exp load across engines.
    DVE slots sit away from qblock boundaries so the normalization chain
    never queues behind them.
  - AV: vh tiles [128, 65] (65th col = ones -> softmax denominator) as
    stationary; accumulated over all 32 key tiles directly in PSUM
    ([65, 512] per head, held across the qblock), one ktg behind scores.
  - Normalize (deferred into the next qblock): denominator row moved to a
    q-on-partitions layout with tiny K=1 matmuls, ONE [128, 8] exact
    reciprocal (128-lane parallel), per-head output projection, 1/den
    applied as a per-partition tensor_scalar at evacuation, heads summed
    on DVE, bf16 partials DMA'd out. norm_a runs right after the qblock's
    last AV matmul (releasing the av PSUM banks); out-projection subtiles
    spread over even ktg slots of the next qblock.

Host gathers: out[b] = sum of the 4 per-core partials + bv @ Wo.T + bo.
"""

from collections import defaultdict

import ml_dtypes
import numpy as np

import concourse.mybir as mybir
import concourse.tile as tile
from concourse import bacc
from concourse.bass_utils import run_bass_kernel_spmd

F32 = mybir.dt.float32
BF16 = mybir.dt.bfloat16
I16 = mybir.dt.int16
EXP = mybir.ActivationFunctionType.Exp
ADD = mybir.AluOpType.add
MULT = mybir.AluOpType.mult
NPBF16 = ml_dtypes.bfloat16

B, S, D, H = 2, 4096, 512, 8
DK = D // H          # 64
HPC = 2              # heads per core
HD = HPC * DK        # 128 head-dims per core
N_CORES = 8
QB = 512             # query block (matmul free dim)
KT = 128             # key tile (partition dim)
NCH = D // 128       # 4 contraction chunks for the projections
KPG = 2              # key tiles per score/exp group
SCW = KPG * QB       # score tile width (1024)
NSUB = QB // 128     # out-projection subtiles per qblock (4)

# Schraudolph bf16 exp: exp(s) ~= bitcast_bf16(int16(s*C1 + C2)); C1 = 2^7/ln2,
# C2 = 127*128 - sigma with sigma tuned for truncating f32->int16 conversion.
EXP_C1 = 184.6649652337873
EXP_C2 = 16251.0
# ktg slots whose exp runs on DVE instead of ACT: ktg % DVE_MOD == 1
DVE_MOD = 4


def mha_tile_kernel(tc, out_ap, ins, seq=S, dve_mod=DVE_MOD):
    nc = tc.nc
    nqb, nkt = seq // QB, seq // KT
    nktg = nkt // KPG
    nst = seq // 128                      # 128-wide s-subtiles for V
    vgroups = [range(g, min(g + 8, nst)) for g in range(0, nst, 8)]

    xq, xk, xv = ins["qt"], ins["kt"], ins["vt"]
    const = tc.alloc_tile_pool(name="const", bufs=1)
    sb = tc.alloc_tile_pool(name="sb", bufs=2)
    scp = tc.alloc_tile_pool(name="scp", bufs=3, space="PSUM")
    avp = tc.alloc_tile_pool(name="avp", bufs=2, space="PSUM")

    # --- weights / constants ---
    wq_sb = const.tile([128, NCH, 128], BF16, tag="wq", name="wq_sb")
    wk_sb = const.tile([128, NCH, 128], BF16, tag="wk", name="wk_sb")
    wv_sb = const.tile([128, NCH, 128], BF16, tag="wv", name="wv_sb")
    wo0_sb = const.tile([64, QB], BF16, tag="wo0", name="wo0_sb")
    wo1_sb = const.tile([64, QB], BF16, tag="wo1", name="wo1_sb")
    bq_sb = const.tile([128, 1], F32, tag="bq", name="bq_sb")
    bk_sb = const.tile([128, 1], F32, tag="bk", name="bk_sb")
    ones_sb = const.tile([128, 64], F32, tag="ones", name="ones_sb")
    nc.vector.memset(ones_sb, 1.0)

    # --- persistent activations ---
    qhT = const.tile([128, seq], BF16, tag="qhT", name="qhT")
    khT = const.tile([128, seq], BF16, tag="khT", name="khT")
    vh = [
        const.tile([128, nkt * 65], BF16, tag=f"vh{h}", name=f"vh{h}")
        for h in range(HPC)
    ]
    for h in range(HPC):
        ones_col = vh[h].rearrange("p (j c) -> p j c", c=65)[:, :, 64]
        nc.vector.tensor_copy(out=ones_col, in_=ones_sb[:, 0:nkt])

    # --- raw inputs in SBUF; DMA'd in [128, 512] slices in consumption order
    xk_sb = [const.tile([128, seq], BF16, tag=f"xk{c}", name=f"xk{c}") for c in range(NCH)]
    xq_sb = [const.tile([128, seq], BF16, tag=f"xq{c}", name=f"xq{c}") for c in range(NCH)]
    xv_sb = [const.tile([128, seq], BF16, tag=f"xv{c}", name=f"xv{c}") for c in range(NCH)]

    def dma_x(dst_tiles, src, j):
        sl = slice(j * QB, (j + 1) * QB)
        for c in range(NCH):
            nc.sync.dma_start(dst_tiles[c][:, sl], src[c * 128 : (c + 1) * 128, sl])

    # DMAs issue serially from the Sync engine (~640ns each after a ~6.6us
    # preamble), so emission order = arrival order. K0's inputs go first,
    # then Q0's, then V-group0's, then the output-projection weights.
    # Weights are host-permuted to [p, c, m] so their DMA is contiguous.
    nvg0 = (len(vgroups[0]) + 3) // 4   # 512-slices covering v-group 0
    nc.sync.dma_start(wk_sb, ins["wk"].rearrange("p (c m) -> p c m", m=128))
    nc.sync.dma_start(bk_sb, ins["bk"])
    dma_x(xk_sb, xk, 0)
    nc.sync.dma_start(wq_sb, ins["wq"].rearrange("p (c m) -> p c m", m=128))
    nc.sync.dma_start(bq_sb, ins["bq"])
    dma_x(xq_sb, xq, 0)
    nc.sync.dma_start(wv_sb, ins["wv"].rearrange("p (c m) -> p c m", m=128))
    for j in range(nvg0):
        dma_x(xv_sb, xv, j)
    nc.sync.dma_start(wo0_sb, ins["wo0"])
    nc.sync.dma_start(wo1_sb, ins["wo1"])
    # remaining K blocks and V groups ordered by their first-use ktg slot in
    # qb0 (K block j injected at ktg 2j-2, V group g at ktg 4g-1), so the PE
    # never waits on an input slice that was queued behind later-needed data
    rest = []
    for j in range(1, nqb):
        rest.append((2 * j - 2, "k", [j]))
    for g in range(1, len(vgroups)):
        rest.append((4 * g - 1, "v", [s for s in (2 * g, 2 * g + 1) if s < nqb]))
    for _, kind, idxs in sorted(rest, key=lambda r: r[0]):
        for j in idxs:
            dma_x(xk_sb if kind == "k" else xv_sb, xk if kind == "k" else xv, j)
    for j in range(1, nqb):
        dma_x(xq_sb, xq, j)

    # --- projection emitters ---
    def proj_block(x_sb, w_sb, bias, dstT, j):
        qsl = slice(j * QB, (j + 1) * QB)
        acc = scp.tile([128, SCW], F32, tag="sc", name=f"prj{j}")[:, 0:QB]
        for c in range(NCH):
            nc.tensor.matmul(
                acc, lhsT=w_sb[:, c, :], rhs=x_sb[c][:, qsl],
                start=(c == 0), stop=(c == NCH - 1),
            )
        if bias is None:
            nc.vector.tensor_copy(out=dstT[:, qsl], in_=acc)
        else:
            nc.vector.tensor_scalar(dstT[:, qsl], acc, bias[:, 0:1], None, ADD)

    def vproj_group(g):
        sts = vgroups[g]
        acc = scp.tile([128, SCW], F32, tag="sc", name=f"vprj{g}")
        for i, st in enumerate(sts):
            a = acc[:, i * 128 : (i + 1) * 128]
            for c in range(NCH):
                nc.tensor.matmul(
                    a, lhsT=xv_sb[c][:, st * 128 : (st + 1) * 128],
                    rhs=wv_sb[:, c, :],
                    start=(c == 0), stop=(c == NCH - 1),
                )
        av_view = acc.rearrange("p (i m) -> p i m", m=128)[:, 0 : len(sts), :]
        for h in range(HPC):
            dst = vh[h].rearrange("p (j c) -> p j c", c=65)[
                :, sts[0] : sts[0] + len(sts), 0:64
            ]
            nc.vector.tensor_copy(out=dst, in_=av_view[:, :, h * 64 : (h + 1) * 64])

    # --- injection schedule: projections emitted inside qb0's ktg loop ---
    inj = defaultdict(list)
    for j in range(1, nqb):                       # K block j needed at ktg 2j
        inj[(0, max(0, min(2 * j - 2, nktg - 1)))].append(("K", j))
    for g in range(1, len(vgroups)):              # V group g needed at ktg 4g+1
        inj[(0, max(0, min(4 * g - 1, nktg - 1)))].append(("V", g))
    for qb in range(nqb - 1):                     # Q block qb+1 inside qb
        inj[(qb, max(0, nktg - 4))].append(("Q", qb + 1))

    # upfront: K block0, Q block0, V group0
    proj_block(xk_sb, wk_sb, bk_sb, khT, 0)
    proj_block(xq_sb, wq_sb, bq_sb, qhT, 0)
    vproj_group(0)

    def emit_av(av_t, ktg, ex):
        for part in range(KPG):
            kt_i = KPG * ktg + part
            for h in range(HPC):
                nc.tensor.matmul(
                    av_t[h][0:65, :],
                    lhsT=vh[h][:, kt_i * 65 : kt_i * 65 + 65],
                    rhs=ex[h][:, part * QB : (part + 1) * QB],
                    start=(ktg == 0 and part == 0),
                    stop=(ktg == nktg - 1 and part == KPG - 1),
                )

    # --- normalize + output projection (deferred into the next qblock) ---
    def norm_a(nstate):
        """Right after the qblock's last AV matmul: move denominators to a
        q-on-partitions layout (tiny K=1 matmuls), one wide reciprocal, and
        release the av PSUM banks via bf16 copies of the AV values."""
        av_t, st8 = nstate["av"], nstate["st8"]
        den_sb = sb.tile([128, SCW], F32, tag="den", bufs=2, name="den_sb")
        av_sb = []
        for h in range(HPC):
            nc.vector.tensor_copy(
                out=den_sb[64:65, h * QB : (h + 1) * QB], in_=av_t[h][64:65, :]
            )
            a = sb.tile([64, QB], BF16, tag=f"avs{h}", bufs=2, name=f"avs{h}")
            nc.vector.tensor_copy(out=a, in_=av_t[h][0:64, :])
            av_sb.append(a)
        denT = scp.tile([128, SCW], F32, tag="sc", name="denT")
        for h in range(HPC):
            for st in range(NSUB):
                j = h * NSUB + st
                o = h * QB + st * 128
                nc.tensor.matmul(
                    denT[:, j : j + 1],
                    lhsT=den_sb[64:65, o : o + 128],
                    rhs=ones_sb[64:65, 0:1],
                    start=True, stop=True,
                )
        rt_sb = sb.tile([128, 2 * NSUB], F32, tag="rt", bufs=2, name="rt_sb")
        nc.vector.reciprocal(
            out=rt_sb, in_=denT[:, 0 : 2 * NSUB]
        )
        st8["rt"] = rt_sb
        st8["av_sb"] = av_sb

    def norm_b(nstate):
        """One out-projection subtile: per-head K=64 matmuls into op halves,
        per-partition 1/den scale at evacuation, DVE head-sum, DMA out."""
        st = nstate["sub"]
        nstate["sub"] += 1
        qb, st8 = nstate["qb"], nstate["st8"]
        rt_sb, av_sb = st8["rt"], st8["av_sb"]
        ssl = slice(st * 128, (st + 1) * 128)
        op = scp.tile([128, SCW], F32, tag="sc", name="op")
        t_h = []
        for h in range(HPC):
            nc.tensor.matmul(
                op[:, h * QB : (h + 1) * QB],
                lhsT=av_sb[h][:, ssl],
                rhs=(wo0_sb if h == 0 else wo1_sb),
                start=True, stop=True,
            )
            t = sb.tile([128, QB], BF16, tag="ot", bufs=4, name=f"ot{h}")
            nc.vector.tensor_scalar(
                t, op[:, h * QB : (h + 1) * QB],
                rt_sb[:, h * NSUB + st : h * NSUB + st + 1], None, MULT,
            )
            t_h.append(t)
        ost = sb.tile([128, QB], BF16, tag="ost", bufs=3, name="ost")
        nc.vector.tensor_tensor(ost, t_h[0], t_h[1], ADD)
        base = qb * QB + st * 128
        nc.sync.dma_start(out_ap[base : base + 128, :], ost)

    # --- attention ---
    nstate = None
    for qb in range(nqb):
        qsl = slice(qb * QB, (qb + 1) * QB)
        av_t = [
            avp.tile([128, QB], F32, tag="av", name=f"av{h}") for h in range(HPC)
        ]
        prev = None
        for ktg in range(nktg):
            for kind, arg in inj.get((qb, ktg), ()):
                if kind == "K":
                    proj_block(xk_sb, wk_sb, bk_sb, khT, arg)
                elif kind == "Q":
                    proj_block(xq_sb, wq_sb, bq_sb, qhT, arg)
                else:
                    vproj_group(arg)
            # scores for both heads (row-tiled K=64 pairs co-execute)
            sc_t = [
                scp.tile([128, SCW], F32, tag="sc", name=f"sc{h}")
                for h in range(HPC)
            ]
            for part in range(KPG):
                kt_i = KPG * ktg + part
                ksl = slice(kt_i * KT, (kt_i + 1) * KT)
                for h in range(HPC):
                    hp = slice(h * 64, (h + 1) * 64)
                    nc.tensor.matmul(
                        sc_t[h][:, part * QB : (part + 1) * QB],
                        lhsT=khT[hp, ksl], rhs=qhT[hp, qsl],
                        start=True, stop=True,
                    )
            # exp: ACT or DVE (Schraudolph) per schedule
            ex = []
            for h in range(HPC):
                if dve_mod and ktg % dve_mod == 1:
                    e = sb.tile([128, SCW], I16, tag="exi", bufs=4, name="exi")
                    nc.vector.tensor_scalar(
                        e[:, :], sc_t[h], EXP_C1, EXP_C2, MULT, ADD
                    )
                    ex.append(e[:, :].bitcast(BF16))
                else:
                    e = sb.tile([128, SCW], BF16, tag="exb", bufs=5, name="exb")
                    nc.scalar.activation(e, sc_t[h], EXP)
                    ex.append(e[:, :])
            # deferred out-projection of the previous qblock: one subtile per
            # even ktg slot (DVE-exp slots are odd, so the evacuation work
            # never queues behind a DVE exp tile)
            if (
                nstate is not None
                and ktg >= 2
                and (ktg - 2) % 4 == 0
                and nstate["sub"] < NSUB
            ):
                norm_b(nstate)
            if prev is not None:
                emit_av(av_t, *prev)
            prev = (ktg, ex)
        emit_av(av_t, *prev)
        if nstate is not None:  # flush any remaining subtiles (small-seq case)
            while nstate["sub"] < NSUB:
                norm_b(nstate)
        # normalization prep right after this qblock's last AV matmul
        nstate = {"av": av_t, "qb": qb, "st8": {}, "sub": 0}
        norm_a(nstate)
    while nstate["sub"] < NSUB:
        norm_b(nstate)

    avp.release()
    scp.release()
    sb.release()
    const.release()


def build_bass(seq=S, dve_mod=DVE_MOD):
    nc = bacc.Bacc(
        "TRN2",
        debug=False,
        enable_asserts=False,
        target_bir_lowering=False,
    )
    ins = {}
    shapes = {
        "qt": (D, seq), "kt": (D, seq), "vt": (D, seq),
        "wq": (128, NCH * HD), "wk": (128, NCH * HD), "wv": (128, NCH * HD),
        "wo0": (64, D), "wo1": (64, D),
        "bq": (HD, 1), "bk": (HD, 1),
    }
    bf16_names = {"qt", "kt", "vt", "wq", "wk", "wv", "wo0", "wo1"}
    for name, shape in shapes.items():
        dt = BF16 if name in bf16_names else F32
        ins[name] = nc.dram_tensor(name, list(shape), dt, kind="ExternalInput").ap()
    out = nc.dram_tensor("out", [seq, D], BF16, kind="ExternalOutput").ap()
    with tile.TileContext(nc) as tc:
        mha_tile_kernel(tc, out, ins, seq=seq, dve_mod=dve_mod)
    nc.compile()
    return nc


def shard_inputs(q, k, v, Wq, bq, Wk, bk, Wv, bv, Wo, bo, seq=S):
    """Host-side shard prep. Returns (in_maps, const_vec)."""
    scale = 1.0 / np.sqrt(np.float32(DK))
    q, k, v = (np.asarray(x, np.float32) for x in (q, k, v))
    Wq, bq, Wk, bk, Wv, bv, Wo, bo = (
        np.asarray(x, np.float32) for x in (Wq, bq, Wk, bk, Wv, bv, Wo, bo)
    )
    bf = lambda x: np.ascontiguousarray(x).astype(NPBF16)
    # device weight layout [p, c*m]: w_dev[p, c, m] = w.T[c*128 + p, m]
    wperm = lambda w: (
        w.T.reshape(NCH, 128, HD).transpose(1, 0, 2).reshape(128, NCH * HD)
    )
    in_maps = []
    for c in range(N_CORES):
        b = c // 4
        rows = slice(128 * (c % 4), 128 * (c % 4) + 128)
        in_maps.append({
            "qt": bf(q[b].T),
            "kt": bf(k[b].T),
            "vt": bf(v[b].T),
            "wq": bf(wperm(Wq[rows, :] * scale)),
            "wk": bf(wperm(Wk[rows, :])),
            "wv": bf(wperm(Wv[rows, :])),
            "wo0": bf(Wo[:, rows][:, 0:64].T),
            "wo1": bf(Wo[:, rows][:, 64:128].T),
            "bq": np.ascontiguousarray((bq[rows] * scale).reshape(HD, 1)),
            "bk": np.ascontiguousarray(bk[rows].reshape(HD, 1)),
        })
    const_vec = (bv @ Wo.T + bo).astype(np.float32)
    return in_maps, const_vec


_NC_CACHE = {}


def run(inputs, seq=S, trace=False, trace_kwargs=None):
    if seq not in _NC_CACHE:
        _NC_CACHE[seq] = build_bass(seq=seq)
    nc = _NC_CACHE[seq]
    in_maps, const_vec = shard_inputs(**inputs, seq=seq)
    res = run_bass_kernel_spmd(
        nc,
        in_maps,
        core_ids=list(range(N_CORES)),
        trace=trace,
        **(trace_kwargs or {}),
    )
    out = np.zeros((B, seq, D), dtype=np.float32)
    for c in range(N_CORES):
        out[c // 4] += np.asarray(res.results[c]["out"], dtype=np.float32)
    out += const_vec[None, None, :]
    return out, res


def kernel(**inputs):
    out, _ = run(inputs)
    return out
